# revision 48
# baseline (speedup 1.0000x reference)
"""Trainium2 Bass kernel for HIVNet GCN message passing (8-core SPMD).

Strategy (v3 — transposed dense aggregation, table-stationary):
  - Pad N=10000 nodes to 10240 = 80 blocks x 128; core c owns 10 dst-blocks
    (1280 nodes).  Node state h is kept TRANSPOSED: hT[half][h, n] with the
    hidden dim on partitions (2 halves of 128) and the core's 1280 nodes on
    the free axis.
  - Per layer: GEMM hws = hs @ W[l] produces node-major [128, 256] blocks
    (lhsT = hsT directly, no transposes), cast to fp8e4m3 (x32), AllGather'd
    into a DRAM table; remote shards are loaded into SBUF.
  - Aggregation (TensorE, DoubleRow fp8): stationary = table block-pair
    [128, 2, 128-H-half], moving = host-built dense edge-count matrix
    sel[p, i, dst] over the core's 1280 dst in 512-wide chunks.  psHT[half]
    [128, 1280] accumulates over all 40 pairs; the core's OWN 5 pairs read
    the local fp8 copy and overlap the AllGather of the rest.
  - BN: fused DVE tensor_tensor_reduce produces t = ps*nrm/S (+sum) and
    t^2 (+sumsq) in two passes; [128, 4] AllReduce; a,c are per-partition so
    apply is a single fused ACT Relu(a*t + c) per half + residual add.
  - Readout: transpose h once, graph mean-pool one-hots (1/cnt folded),
    2*128-row AllReduce, 3-layer MLP.
"""

import sys

sys.path.insert(0, "/opt/trn_rl_repo")

from contextlib import ExitStack

import numpy as np
import ml_dtypes

from concourse import bass, mybir, bacc, tile
from concourse.bass_utils import run_bass_kernel_spmd
from concourse.masks import make_identity

NCORE = 8
P = 128
H = 256
L = 4
NF = 9
G = 256
N = 10000
BPC = 10                # dst blocks per core
NPC = BPC * P           # 1280 nodes per core
NPAD = NCORE * NPC      # 10240
NBLK = NPAD // P        # 80 src blocks
NPAIR = NBLK // 2       # 40 src block pairs (DoubleRow K=256)
BN_EPS = 1e-5
FP8S = 32.0             # fp8 table scale
CHUNKS = [(0, 512), (512, 512), (1024, 256)]   # dst chunks (<=512 f32 PSUM bank)

f32 = mybir.dt.float32
bf16 = mybir.dt.bfloat16
fp8 = mybir.dt.float8e4
fp8np = mybir.dt.np(mybir.dt.float8e4)
bfnp = ml_dtypes.bfloat16

FT = mybir.ActivationFunctionType
OP = mybir.AluOpType
DRM = mybir.MatmulPerfMode.DoubleRow

_compiled = {}


# --------------------------------------------------------------------------
# host-side structural preprocessing
# --------------------------------------------------------------------------

def _preprocess(x, edge_index, batch_ids, emb, W, gamma, beta,
                mlp_W1, mlp_b1, mlp_W2, mlp_b2, mlp_W3, mlp_b3):
    src = np.asarray(edge_index[0], np.int64)
    dst = np.asarray(edge_index[1], np.int64)
    src_all = np.concatenate([src, np.arange(N, dtype=np.int64)])
    dst_all = np.concatenate([dst, np.arange(N, dtype=np.int64)])

    deg = np.bincount(dst_all, minlength=NPAD).astype(np.float64)
    nrm_full = np.zeros(NPAD, np.float32)
    nrm_full[:NPAD] = 1.0 / np.sqrt(np.maximum(deg, 1.0))
    nrm_full[deg == 0] = 0.0

    # dense per-core sel (moving operand): [128 p, 40 pair, 2 i, 1280 dst]
    p_idx = (src_all % P).astype(np.int64)
    blk = src_all // P
    k_idx = blk // 2
    i_idx = blk % 2
    core = dst_all // NPC
    d_idx = dst_all % NPC
    sels = []
    for c in range(NCORE):
        m = core == c
        selc = np.zeros((P, NPAIR, 2, NPC), np.float32)
        np.add.at(selc, (p_idx[m], k_idx[m], i_idx[m], d_idx[m]), 1.0)
        sels.append(selc.reshape(P, NPAIR * 2 * NPC).astype(fp8np))

    # graph pool one-hot with 1/count folded in
    bids = np.asarray(batch_ids, np.int64)
    cnt = np.bincount(bids, minlength=G).astype(np.float32)
    inv = 1.0 / np.maximum(cnt, 1.0)
    psel_full = np.zeros((NPAD, G), np.float32)
    psel_full[np.arange(N), bids] = inv[bids]

    x_np = np.zeros((NPAD, NF), np.float32)
    x_np[:N] = np.asarray(x, np.float64)

    # encoder prep on host: D = emb1 - emb0, base = sum_f emb0[f]
    embf = np.asarray(emb, np.float32)
    D = np.ascontiguousarray(embf[:, 1, :] - embf[:, 0, :])       # [9, 256]
    baseT = np.ascontiguousarray(embf[:, 0, :].sum(0).reshape(2, P).T)  # [128,2]

    Wf = np.asarray(W, np.float32)
    W_lhsT = Wf.reshape(L, 2, P, H).transpose(2, 0, 1, 3).reshape(P, L * 2 * H)
    # gamma/beta transposed per half: [128, L*4] = (g0,g1,b0,b1) per layer
    gaT = np.asarray(gamma, np.float32).reshape(L, 2, P)
    beT = np.asarray(beta, np.float32).reshape(L, 2, P)
    gbT = np.concatenate([gaT, beT], axis=1).transpose(2, 0, 1).reshape(P, L * 4)

    w1 = np.asarray(mlp_W1, np.float32).reshape(2, P, P).transpose(1, 0, 2).reshape(P, 2 * P)
    w2 = np.asarray(mlp_W2, np.float32)
    w3 = np.asarray(mlp_W3, np.float32)
    b1 = np.asarray(mlp_b1, np.float32).reshape(P, 1)
    b2 = np.asarray(mlp_b2, np.float32).reshape(64, 1)
    b3 = np.asarray(mlp_b3, np.float32).reshape(1, 1)

    in_maps = []
    for c in range(NCORE):
        lo, hi = c * NPC, (c + 1) * NPC
        nrmc = nrm_full[lo:hi]
        nrm_exp = np.broadcast_to(nrmc, (P, NPC)).copy()          # [128,1280]
        nrms_exp = (nrm_exp / FP8S).astype(np.float32)
        # node-major per-block scale for the GEMM-output fp8 cast
        nrmS = np.ascontiguousarray(nrmc.reshape(BPC, P).T * FP8S)  # [128,BPC]

        pselc = psel_full[lo:hi].reshape(BPC, P, G)
        pselc = np.ascontiguousarray(pselc.transpose(1, 0, 2)).reshape(P, BPC * G)

        xTc = np.ascontiguousarray(x_np[lo:hi].T)                 # [9, 1280]

        in_maps.append(dict(
            selw=sels[c], xT=xTc, psel=pselc,
            nrme=nrm_exp, nrmse=nrms_exp, nrmS=nrmS,
            W=W_lhsT.astype(bfnp), gbT=gbT, D=D, baseT=baseT,
            w1=w1, w2=w2, w3=w3, b1=b1, b2=b2, b3=b3,
        ))
    return in_maps


# --------------------------------------------------------------------------
# device program
# --------------------------------------------------------------------------

def _build():
    SELW = NPAIR * 2 * NPC          # 102400
    nc = bacc.Bacc(None, target_bir_lowering=False)

    d_sel = nc.dram_tensor("selw", [P, SELW], fp8, kind="ExternalInput")
    d_xT = nc.dram_tensor("xT", [NF, NPC], f32, kind="ExternalInput")
    d_psel = nc.dram_tensor("psel", [P, BPC * G], f32, kind="ExternalInput")
    d_nrme = nc.dram_tensor("nrme", [P, NPC], f32, kind="ExternalInput")
    d_nrmse = nc.dram_tensor("nrmse", [P, NPC], f32, kind="ExternalInput")
    d_nrmS = nc.dram_tensor("nrmS", [P, BPC], f32, kind="ExternalInput")
    d_W = nc.dram_tensor("W", [P, L * 2 * H], bf16, kind="ExternalInput")
    d_gbT = nc.dram_tensor("gbT", [P, L * 4], f32, kind="ExternalInput")
    d_D = nc.dram_tensor("D", [NF, H], f32, kind="ExternalInput")
    d_baseT = nc.dram_tensor("baseT", [P, 2], f32, kind="ExternalInput")
    d_w1 = nc.dram_tensor("w1", [P, 2 * P], f32, kind="ExternalInput")
    d_w2 = nc.dram_tensor("w2", [P, 64], f32, kind="ExternalInput")
    d_w3 = nc.dram_tensor("w3", [64, 1], f32, kind="ExternalInput")
    d_b1 = nc.dram_tensor("b1", [P, 1], f32, kind="ExternalInput")
    d_b2 = nc.dram_tensor("b2", [64, 1], f32, kind="ExternalInput")
    d_b3 = nc.dram_tensor("b3", [1, 1], f32, kind="ExternalInput")
    d_out = nc.dram_tensor("out", [1, G], f32, kind="ExternalOutput")

    rg = [list(range(NCORE))]

    with tile.TileContext(nc) as tc, ExitStack() as ctx:
        pers = ctx.enter_context(tc.tile_pool(name="pers", bufs=1))
        psH = ctx.enter_context(tc.tile_pool(name="psH", bufs=1, space="PSUM"))
        psB = ctx.enter_context(tc.tile_pool(name="psB", bufs=1, space="PSUM"))
        work = ctx.enter_context(tc.tile_pool(name="work", bufs=2))
        stream = ctx.enter_context(tc.tile_pool(name="stream", bufs=2))
        dram = ctx.enter_context(tc.tile_pool(name="dram", bufs=2, space="DRAM"))

        # ---- persistent SBUF state -------------------------------------
        sel_sb = pers.tile([P, SELW], fp8, tag="sel")
        tab_sb = pers.tile([P, NBLK * H], fp8, tag="tab")
        nrme_sb = pers.tile([P, NPC], f32, tag="nrme")
        nrmse_sb = pers.tile([P, NPC], f32, tag="nrmse")
        nrmS_sb = pers.tile([P, BPC], f32, tag="nrmS")
        W_sb = pers.tile([P, L * 2 * H], bf16, tag="W")
        gbT_sb = pers.tile([P, L * 4], f32, tag="gbT")
        D_sb = pers.tile([NF, H], f32, tag="D")
        baseT_sb = pers.tile([P, 2], f32, tag="baseT")
        w1_sb = pers.tile([P, 2 * P], f32, tag="w1")
        w2_sb = pers.tile([P, 64], f32, tag="w2")
        w3_sb = pers.tile([64, 1], f32, tag="w3")
        b1_sb = pers.tile([P, 1], f32, tag="b1")
        b2_sb = pers.tile([64, 1], f32, tag="b2")
        b3_sb = pers.tile([1, 1], f32, tag="b3")

        hT_sb = pers.tile([P, 2 * NPC], f32, tag="hT")      # halves side by side
        tT_sb = pers.tile([P, 2 * NPC], f32, tag="tT")
        sq_sb = pers.tile([P, NPC], f32, tag="sqs")         # scratch for t^2
        hsT_sb = pers.tile([P, 2 * NPC], bf16, tag="hsT")
        ag_sb = pers.tile([P, BPC * H], fp8, tag="ag")
        stat_sb = pers.tile([P, 4], f32, tag="stat")
        ac_sb = pers.tile([P, 8], f32, tag="ac")            # mu0 mu1 a0 a1 c0 c1 tmp
        ident_f = pers.tile([P, P], f32, tag="ident")

        # ---- DRAM bounce buffers ---------------------------------------
        B1 = 4                 # split AllGather: first 4 blocks / last 6
        ag_in1 = dram.tile([P, B1 * H], fp8, tag="ag_in1")
        ag_in2 = dram.tile([P, (BPC - B1) * H], fp8, tag="ag_in2")
        ag1_outs = [dram.tile([NCORE * P, B1 * H], fp8, tag=f"ag1_out{l}",
                              addr_space="Shared", name=f"ag1_out{l}")
                    for l in range(L)]
        ag2_outs = [dram.tile([NCORE * P, (BPC - B1) * H], fp8,
                              tag=f"ag2_out{l}", addr_space="Shared",
                              name=f"ag2_out{l}")
                    for l in range(L)]

        ar_in = dram.tile([P, 4], f32, tag="ar_in")
        ar_outs = [dram.tile([P, 4], f32, tag=f"ar_out{l}",
                             addr_space="Shared", name=f"ar_out{l}")
                   for l in range(L)]
        pr_in = dram.tile([2 * P, G], f32, tag="pr_in")
        pr_out = dram.tile([2 * P, G], f32, tag="pr_out", addr_space="Shared")

        # ---- input loads (small first; 13MB sel last) -------------------
        xT_sb = stream.tile([NF, NPC], f32, tag="xT_sb")
        nc.sync.dma_start(out=xT_sb[:], in_=d_xT[:])
        for t, d in [(nrme_sb, d_nrme), (nrmse_sb, d_nrmse),
                     (nrmS_sb, d_nrmS), (W_sb, d_W),
                     (gbT_sb, d_gbT), (D_sb, d_D), (baseT_sb, d_baseT),
                     (w1_sb, d_w1), (w2_sb, d_w2), (w3_sb, d_w3),
                     (b1_sb, d_b1), (b2_sb, d_b2), (b3_sb, d_b3)]:
            nc.sync.dma_start(out=t[:], in_=d[:])
        # 13MB sel load rides the scalar HWDGE ring so it doesn't block the
        # sync ring (ag_in upload, tab section loads) during layer 0.
        nc.scalar.dma_start(out=sel_sb[:], in_=d_sel[:])
        make_identity(nc, ident_f[:])

        def hT(half):
            return hT_sb[:, half * NPC:(half + 1) * NPC]

        def tT(half):
            return tT_sb[:, half * NPC:(half + 1) * NPC]

        def hsT(half):
            return hsT_sb[:, half * NPC:(half + 1) * NPC]

        # ---- encoder: hT = D^T @ xT + baseT -----------------------------
        psHT = [psH.tile([P, NPC], f32, tag=f"h{i}", name=f"psHT{i}")
                for i in range(2)]
        for half in range(2):
            for (off, ln) in CHUNKS:
                nc.tensor.matmul(out=psHT[half][:, off:off + ln],
                                 lhsT=D_sb[:, half * P:(half + 1) * P],
                                 rhs=xT_sb[:, off:off + ln],
                                 start=True, stop=True)
            nc.vector.tensor_scalar_add(hT(half), psHT[half][:],
                                        baseT_sb[:, half:half + 1])
            # bf16 copy of h for the GEMM (nrm folds into the fp8 cast)
            nc.vector.tensor_copy(out=hsT(half), in_=hT(half))

        # ---- layers -----------------------------------------------------
        for l in range(L):
            # GEMM hws = hs @ W[l] per dst block, cast to fp8 table shard
            for nb in range(BPC):
                ps_g = psB.tile([P, H], f32, tag="mm" if nb % 2 == 0 else "mm2",
                                name=f"ps_g{nb}")
                for half in range(2):
                    nc.tensor.matmul(
                        out=ps_g[:],
                        lhsT=hsT(half)[:, nb * P:(nb + 1) * P],
                        rhs=W_sb[:, (l * 2 + half) * H:(l * 2 + half + 1) * H],
                        start=(half == 0), stop=(half == 1))
                nc.scalar.activation(out=ag_sb[:, nb * H:(nb + 1) * H],
                                     in_=ps_g[:], func=FT.Copy,
                                     scale=nrmS_sb[:, nb:nb + 1])
            nc.sync.dma_start(out=ag_in1[:], in_=ag_sb[:, 0:B1 * H])
            nc.gpsimd.collective_compute(
                "AllGather", OP.bypass, replica_groups=rg,
                ins=[ag_in1[:]], outs=[ag1_outs[l][:]])
            nc.sync.dma_start(out=ag_in2[:], in_=ag_sb[:, B1 * H:])
            nc.gpsimd.collective_compute(
                "AllGather", OP.bypass, replica_groups=rg,
                ins=[ag_in2[:]], outs=[ag2_outs[l][:]])
            for r in range(NCORE):
                nc.sync.dma_start(
                    out=tab_sb[:, (r * BPC) * H:(r * BPC + B1) * H],
                    in_=ag1_outs[l][r * P:(r + 1) * P, :])
            for r in range(NCORE):
                nc.sync.dma_start(
                    out=tab_sb[:, (r * BPC + B1) * H:(r + 1) * BPC * H],
                    in_=ag2_outs[l][r * P:(r + 1) * P, :])

            # aggregation: psHT[half] += tab_pair^T (DR) @ sel chunks.
            # half 0 finishes all pairs first so its stats (DVE) overlap
            # half 1's matmuls.
            psHT = [psH.tile([P, NPC], f32, tag=f"h{i}", name=f"psT{l}{i}")
                    for i in range(2)]
            # pairs covered by AG1 (first 2 pairs of each rank) first, so
            # aggregation starts while AG2 is still in flight
            korder = [k for k in range(NPAIR) if k % 5 < B1 // 2] + \
                     [k for k in range(NPAIR) if k % 5 >= B1 // 2]
            for half in range(2):
                for ki, k in enumerate(korder):
                    tpair = tab_sb[:, 2 * k * H:(2 * k + 2) * H].rearrange(
                        "p (two h) -> p two h", two=2)
                    spair = sel_sb[:, k * 2 * NPC:(k + 1) * 2 * NPC].rearrange(
                        "p (two d) -> p two d", two=2)
                    lhsT = tpair[:, :, half * P:(half + 1) * P]
                    for (off, ln) in CHUNKS:
                        nc.tensor.matmul(
                            out=psHT[half][:, off:off + ln],
                            lhsT=lhsT,
                            rhs=spair[:, :, off:off + ln],
                            start=(ki == 0), stop=(ki == NPAIR - 1),
                            perf_mode=DRM)
                # t = ps*nrm/S (+col-sum), sq = t*t (+col-sumsq) — fused DVE
                nc.vector.scalar_tensor_tensor(
                    out=tT(half), in0=psHT[half][:], scalar=1.0,
                    in1=nrmse_sb[:], op0=OP.mult, op1=OP.mult,
                    accum_out=stat_sb[:, half:half + 1])
                nc.vector.scalar_tensor_tensor(
                    out=sq_sb[:], in0=tT(half), scalar=1.0,
                    in1=tT(half), op0=OP.mult, op1=OP.mult,
                    accum_out=stat_sb[:, 2 + half:3 + half])
            nc.sync.dma_start(out=ar_in[:], in_=stat_sb[:])
            nc.gpsimd.collective_compute(
                "AllReduce", OP.add, replica_groups=rg,
                ins=[ar_in[:]], outs=[ar_outs[l][:]])
            nc.sync.dma_start(out=stat_sb[:], in_=ar_outs[l][:])

            # per-partition BN coeffs: a = gamma*istd, c = beta - mu*a
            mu2 = ac_sb[:, 0:2]
            var2 = ac_sb[:, 2:4]
            a2 = ac_sb[:, 4:6]
            c2 = ac_sb[:, 6:8]
            nc.vector.tensor_scalar_mul(mu2, stat_sb[:, 0:2], 1.0 / N)
            nc.vector.tensor_scalar_mul(var2, stat_sb[:, 2:4], 1.0 / N)
            nc.vector.tensor_tensor(out=a2, in0=mu2, in1=mu2, op=OP.mult)
            nc.vector.tensor_tensor(out=var2, in0=var2, in1=a2, op=OP.subtract)
            nc.vector.tensor_scalar_add(var2, var2, BN_EPS)
            nc.vector.reciprocal(out=var2, in_=var2)
            nc.scalar.activation(out=var2, in_=var2, func=FT.Sqrt)  # istd
            nc.vector.tensor_tensor(out=a2, in0=var2,
                                    in1=gbT_sb[:, l * 4:l * 4 + 2], op=OP.mult)
            nc.vector.tensor_tensor(out=c2, in0=mu2, in1=a2, op=OP.mult)
            nc.vector.tensor_tensor(out=c2, in0=gbT_sb[:, l * 4 + 2:l * 4 + 4],
                                    in1=c2, op=OP.subtract)

            # h += relu(a*t + c) ; hs = h*nrm for next GEMM.  Split into two
            # node-chunks so the next layer's GEMM (which consumes hs slices
            # per 128-node block) can start while the second chunk applies.
            for (off, ln) in [(0, 5 * P), (5 * P, 5 * P)]:
                for half in range(2):
                    r_t = work.tile([P, ln], f32, tag="r_t")
                    nc.scalar.activation(out=r_t[:],
                                         in_=tT(half)[:, off:off + ln],
                                         func=FT.Relu,
                                         scale=ac_sb[:, 4 + half:5 + half],
                                         bias=ac_sb[:, 6 + half:7 + half])
                    nc.vector.tensor_tensor(out=hT(half)[:, off:off + ln],
                                            in0=hT(half)[:, off:off + ln],
                                            in1=r_t[:], op=OP.add)
                    if l < L - 1:
                        nc.vector.tensor_copy(
                            out=hsT(half)[:, off:off + ln],
                            in_=hT(half)[:, off:off + ln])

        # ---- pooling: transpose hT blocks, one-hot matmul ---------------
        ps_p0 = psB.tile([P, G], f32, tag="mm")
        ps_p1 = psB.tile([P, G], f32, tag="mm2")
        hblk = [work.tile([P, P], f32, tag=f"hp{i}", name=f"hblk{i}")
                for i in range(2)]
        for nb in range(BPC):
            psel_t = stream.tile([P, G], f32, tag="psel_t")
            nc.sync.dma_start(out=psel_t[:], in_=d_psel[:, nb * G:(nb + 1) * G])
            for half in range(2):
                ps_tr = psH.tile([P, P], f32, tag=f"h{half}", name=f"ptr{half}")
                nc.tensor.transpose(out=ps_tr[:],
                                    in_=hT(half)[:, nb * P:(nb + 1) * P],
                                    identity=ident_f[:])
                nc.vector.tensor_copy(out=hblk[half][:], in_=ps_tr[:])
            nc.tensor.matmul(out=ps_p0[:], lhsT=hblk[0][:], rhs=psel_t[:],
                             start=(nb == 0), stop=(nb == BPC - 1))
            nc.tensor.matmul(out=ps_p1[:], lhsT=hblk[1][:], rhs=psel_t[:],
                             start=(nb == 0), stop=(nb == BPC - 1))
        g0 = work.tile([P, G], f32, tag="g0")
        g1 = work.tile([P, G], f32, tag="g1")
        nc.vector.tensor_copy(out=g0[:], in_=ps_p0[:])
        nc.vector.tensor_copy(out=g1[:], in_=ps_p1[:])
        nc.sync.dma_start(out=pr_in[0:P, :], in_=g0[:])
        nc.sync.dma_start(out=pr_in[P:2 * P, :], in_=g1[:])
        nc.gpsimd.collective_compute(
            "AllReduce", OP.add, replica_groups=rg,
            ins=[pr_in[:]], outs=[pr_out[:]])
        nc.sync.dma_start(out=g0[:], in_=pr_out[0:P, :])
        nc.sync.dma_start(out=g1[:], in_=pr_out[P:2 * P, :])

        # MLP head (weights as lhsT, graphs along free dim)
        ps1 = psB.tile([P, G], f32, tag="mm")
        nc.tensor.matmul(out=ps1[:], lhsT=w1_sb[:, 0:P], rhs=g0[:],
                         start=True, stop=False)
        nc.tensor.matmul(out=ps1[:], lhsT=w1_sb[:, P:2 * P], rhs=g1[:],
                         start=False, stop=True)
        y1 = work.tile([P, G], f32, tag="y1")
        nc.scalar.activation(out=y1[:], in_=ps1[:], func=FT.Relu,
                             bias=b1_sb[:, 0:1])
        ps2 = psB.tile([64, G], f32, tag="mm2")
        nc.tensor.matmul(out=ps2[:], lhsT=w2_sb[:], rhs=y1[:],
                         start=True, stop=True)
        y2 = work.tile([64, G], f32, tag="y2")
        nc.scalar.activation(out=y2[:], in_=ps2[:], func=FT.Relu,
                             bias=b2_sb[:, 0:1])
        ps3 = psB.tile([1, G], f32, tag="mm")
        nc.tensor.matmul(out=ps3[:], lhsT=w3_sb[:], rhs=y2[:],
                         start=True, stop=True)
        y3 = work.tile([1, G], f32, tag="y3")
        nc.vector.tensor_scalar_add(y3[:], ps3[:], b3_sb[0:1, 0:1])
        nc.sync.dma_start(out=d_out[:], in_=y3[:])

    nc.compile()
    return nc


# --------------------------------------------------------------------------
# entry point
# --------------------------------------------------------------------------

def kernel(x, edge_index, batch_ids, emb, W, b, gamma, beta,
           mlp_W1, mlp_b1, mlp_W2, mlp_b2, mlp_W3, mlp_b3,
           _trace=False, _trace_kwargs=None):
    in_maps = _preprocess(x, edge_index, batch_ids, emb, W, gamma, beta,
                          mlp_W1, mlp_b1, mlp_W2, mlp_b2, mlp_W3, mlp_b3)
    if "nc" not in _compiled:
        _compiled["nc"] = _build()
    nc = _compiled["nc"]
    kw = {}
    if _trace:
        kw = dict(trace=True, **(_trace_kwargs or {}))
    res = run_bass_kernel_spmd(nc, in_maps, core_ids=list(range(NCORE)), **kw)
    out = np.asarray(res.results[0]["out"], np.float32).reshape(G, 1)
    kernel._last_results = res
    return out
